# revision 7
# baseline (speedup 1.0000x reference)
"""Trainium2 Bass kernel for nn_AggregationLayer (pooling) — v2.

Computation (reference):
    fact: [N,H], elements_p: [M,H], weights: [H,H]   (N=32768, M=4096, H=768)
    fact_Q = max(fact, axis=0)                     # [1,H]
    f2e = softmax(fact_Q @ W @ ep.T)               # [1,M]
    ep_ = f2e @ ep                                 # [1,H]
    e2f = softmax_n(fact @ W @ ep_.T)              # [N,1]
    fact_ = e2f.T @ fact                           # [1,H]
    returns (fact_, ep_)

v2 key ideas vs baseline:
  - fact is sent ONCE per layout as fp16 of (fact - 4.0).  The shift makes
    the column max (values ~4.05 +- 0.3) carry ~8x more absolute precision
    in fp16; the e2f logits only change by a constant (softmax-invariant)
    and the weighted row-sum just needs +4.0 added back on the host.
  - The precision-critical f2e chain (fQ -> qw -> l1) stays fp32: w and epT
    are fp32r (full-rate PE streaming for free dims >= 256).
  - factT DMAs straight into its resident [128, HC, NS] SBUF layout; the
    per-chunk column max runs on DVE as chunks land, and the cross-core
    AllReduce(max) is split in two halves pipelined behind the DMA.
  - Everything downstream (matvec, exp weights, weighted sum) is fp16 at
    1 cycle/row on PE.
Distribution: fact sharded along N (4096 rows/core), elements_p along M
(512 rows/core), weights replicated.  AllReduce(max) for fact_Q, AllGather
for the f2e softmax stats; e2f softmax partials combined on host.
"""
import sys

sys.path.insert(0, "/opt/trn_rl_repo")

import numpy as np
import ml_dtypes

import concourse.bass as bass
import concourse.tile as tile
import concourse.mybir as mybir
from concourse import bacc, bass_utils

N, M, H = 32768, 4096, 768
NCORES = 8
NS = N // NCORES          # 4096 fact rows per core
MS = M // NCORES          # 512 ep rows per core
HC = H // 128             # 6 h-chunks of 128
NB = NS // 512            # 8 blocks for e2f matvec
NT = NS // 128            # 32 row-tiles of 128
MQ = MS // 128            # 4 ep row-tiles
SHIFT = 4.0

F32 = mybir.dt.float32
F32R = mybir.dt.float32r
F16 = mybir.dt.float16
RG = [list(range(NCORES))]

_CACHE = {}


def _dummy_out(nc, rows, out_ep, out_st):
    d1 = rows.tile([1, H], F32, tag="rows")
    d2 = rows.tile([1, H + 2], F32, tag="row770")
    nc.vector.memset(d1, 0.0)
    nc.vector.memset(d2, 0.0)
    nc.gpsimd.dma_start(out=out_ep, in_=d1)
    nc.gpsimd.dma_start(out=out_st, in_=d2)


def _body(nc, tc, ctx, factT, factn, epT, epn, w, wt, out_ep, out_st, stage=999, pools=None, sim1=False, fake_cc=False, rep=0):
    par = rep % 2
    if pools is None:
        pools = {}
    def pool(name, **kw):
        if name not in pools:
            pools[name] = ctx.enter_context(tc.tile_pool(name=name, **kw))
        return pools[name]
    dram = pool("dram", bufs=1, space="DRAM")
    big = pool("big", bufs=1)
    rows = pool("rows", bufs=2)
    tiny = pool("tiny", bufs=1)
    psA = pool("psA", bufs=1, space="PSUM")
    psE = pool("psE", bufs=2, space="PSUM")
    psT = pool("psT", bufs=1, space="PSUM")
    if "consts" not in pools:
        ones11 = tiny.tile([1, 1], F16, tag="ones11")
        nc.vector.memset(ones11, 1.0)
        ones_col = tiny.tile([128, 1], F32, tag="ones_col")
        nc.vector.memset(ones_col, 1.0)
        ident_d = nc.inline_tensor(np.eye(128, dtype=np.float32), name="ident128")
        ident_t = tiny.tile([128, 128], F32, tag="ident128")
        nc.sync.dma_start(out=ident_t, in_=ident_d.ap())
        ones_col16 = tiny.tile([128, 1], F16, tag="ones_col16")
        nc.vector.memset(ones_col16, 1.0)
        shift_col = tiny.tile([128, 1], F32, tag="shift_col")
        nc.vector.memset(shift_col, SHIFT)
        pools["consts"] = (ones11, ones_col, ident_t, ones_col16, shift_col)
    ones11, ones_col, ident, ones_col16, shift_col = pools["consts"]

    # ---- fact loads: factT fp16 chunks DMA directly into the resident
    # [128, HC, NS] layout on the SP HWDGE ring; each chunk's column max
    # runs on DVE as it lands.
    # chunks 0-4 double-buffered by rep parity (next rep's DMA + col-max +
    # AllReduce overlap this rep's matvec/wsum); chunk 5 single-buffered
    # (its reload chain hides under the weighted sum) to fit SBUF.
    factT_db = big.tile([128, HC - 1, NS], F16, tag=f"factT{par}")
    factT_sh = big.tile([128, 1, NS], F16, tag="factT_sh")
    def fT(c):
        return factT_db[:, c, :] if c < HC - 1 else factT_sh[:, 0, :]
    factT_d = factT.rearrange("(c p) n -> p c n", p=128)
    fQl = tiny.tile([128, HC], F32, tag=f"fQl{par}")
    _fact_dmas = []
    for c in range(HC):
        d = nc.sync.dma_start(out=fT(c), in_=factT_d[:, c, :])
        _fact_dmas.append(d)
        nc.vector.reduce_max(fQl[:, c : c + 1], fT(c), axis=mybir.AxisListType.X)

    # ---- small input loads on the ACT HWDGE ring, gated behind the first
    # few factT chunks so factT streams at full rate early; then fnorm.
    smw = pool("smw", bufs=1)
    smwt = pool("smwt", bufs=1)
    smept = pool("smept", bufs=1)
    smep = pool("smep", bufs=1)
    w_d = w.rearrange("(c p) j -> p c j", p=128)
    wt_d = wt.rearrange("(c p) j -> p c j", p=128)
    epT_d = epT.rearrange("(c p) m -> p c m", p=128)
    epn_d = epn.rearrange("(j p) h -> p j h", p=128)
    _gate = _fact_dmas[-1].ins
    _prev_small = None
    def _small_dma(dst, src):
        nonlocal _prev_small
        d = nc.scalar.dma_start(out=dst, in_=src)
        if _prev_small is None:
            tile.add_dep_helper(d.ins, _gate, sync=True, reason="factT first")
        else:
            tile.add_dep_helper(d.ins, _prev_small, sync=False, reason="ACT ring order")
        _prev_small = d.ins
        return d
    w_sb = smw.tile([128, HC, H], F32R, tag="w")
    for c in range(HC):
        _small_dma(w_sb[:, c, :], w_d[:, c, :])
    epT_sb = smept.tile([128, HC, MS], F32R, tag="epT")
    _small_dma(epT_sb, epT_d)
    ep_sb = smep.tile([128, MQ, H], F16, tag="ep")
    _small_dma(ep_sb, epn_d)
    wt_sb = smwt.tile([128, HC, H], F16, tag="wt")
    for c in range(HC):
        _small_dma(wt_sb[:, c, :], wt_d[:, c, :])
    # fnorm (row-major fp16 fact) streams last, in quarters so the final
    # weighted sum can start before the whole thing lands.
    fnorm_sb = big.tile([128, NT, H], F16, tag="fnorm")
    _fn_d = factn.rearrange("(j p) h -> p j h", p=128)
    fn_dmas = []
    for q in range(4):
        fn_dmas.append(_small_dma(fnorm_sb[:, q * 8 : (q + 1) * 8, :], _fn_d[:, q * 8 : (q + 1) * 8, :]))

    if stage < 1:
        _dummy_out(nc, rows, out_ep, out_st)
        return
    # ---- PE warmup (clock ramp) against landed factT chunks
    def _warm(n, c=0, off=0):
        for _ in range(n):
            ps_w = psE.tile([1, 512], F32, tag="psE")
            nc.tensor.matmul(ps_w, ones_col16, fT(c)[:, off : off + 512],
                             start=True, stop=True)
    # With factT double-buffered, the next rep's AllReduce overlaps this
    # rep's PE tail, so only a small burst is needed to bridge any gap
    # (plus rep-0 cold start, which the slope timing cancels).
    for k in range(2):
        _warm(1, c=k, off=0)
    _warm(2, c=HC - 1, off=2048)

    if stage < 11:
        _dummy_out(nc, rows, out_ep, out_st)
        return
    # ---- AllReduce(max) of the local column max (single [768] op)
    fQg = tiny.tile([128, HC], F32, tag=f"fQg{par}")
    cc1_in = dram.tile([H], F32, tag=f"cc1i{par}")
    cc1_out = dram.tile([H], F32, tag=f"cc1o{par}")
    nc.gpsimd.dma_start(out=cc1_in[:].rearrange("(c p) -> p c", p=128), in_=fQl)
    if sim1 or fake_cc:
        nc.gpsimd.dma_start(out=cc1_out[:], in_=cc1_in[:])
    else:
        nc.gpsimd.collective_compute(
            "AllReduce", mybir.AluOpType.max, replica_groups=RG,
            ins=[cc1_in.opt()], outs=[cc1_out.opt()],
        )
    nc.gpsimd.dma_start(
        out=fQg, in_=cc1_out[:].rearrange("(c p) -> p c", p=128))
    # undo the host-side shift: fQ = smax + 4.0, in fp32r for the qw matmul
    fQr = tiny.tile([128, HC], F32R, tag=f"fQr{par}")
    nc.scalar.activation(out=fQr, in_=fQg, func=mybir.ActivationFunctionType.Copy,
                         bias=float(SHIFT), scale=1.0)

    if stage < 12:
        _dummy_out(nc, rows, out_ep, out_st)
        return
    regions768 = [(0, 512), (512, 256)]

    # ---- qw = fact_Q @ W  (fp32r at full rate; contraction over h)
    ps_qw = psA.tile([1, H], F32, tag="psA")
    for c in range(HC):
        for n0, nn in regions768:
            nc.tensor.matmul(
                ps_qw[:, n0 : n0 + nn], fQr[:, c : c + 1], w_sb[:, c, n0 : n0 + nn],
                start=(c == 0), stop=(c == HC - 1),
            )
    qw_row = rows.tile([1, H], F32, tag="rows")
    nc.scalar.activation(out=qw_row, in_=ps_qw, func=mybir.ActivationFunctionType.Copy)

    # transpose qw row -> [128, HC] via K=1 matmuls against ones
    # (fp32: tiny, 4 cy/row is fine here)
    ones11_32 = None
    qwT = tiny.tile([128, HC], F32R, tag="qwT")
    ps_t = psT.tile([128, HC], F32, tag="psT")
    for c in range(HC):
        nc.tensor.matmul(ps_t[:, c : c + 1], qw_row[:, c * 128 : (c + 1) * 128],
                         ones_col[0:1, :], start=True, stop=True)
    nc.vector.tensor_copy(qwT, ps_t)

    if stage < 13:
        _dummy_out(nc, rows, out_ep, out_st)
        return
    # ---- f2e local logits [1, MS] (fp32r)
    ps_f2e = psA.tile([1, MS], F32, tag="psA")
    for c in range(HC):
        nc.tensor.matmul(
            ps_f2e, qwT[:, c : c + 1], epT_sb[:, c, :],
            start=(c == 0), stop=(c == HC - 1),
        )
    # local softmax pieces
    cmax_l = tiny.tile([1, 1], F32, tag="cmax_l")
    neg_cmax = tiny.tile([1, 1], F32, tag="neg_cmax")
    nc.vector.reduce_max(cmax_l, ps_f2e, axis=mybir.AxisListType.X)
    nc.vector.tensor_reduce(neg_cmax, ps_f2e, axis=mybir.AxisListType.X,
                            op=mybir.AluOpType.max, negate=True)
    p_loc = rows.tile([1, MS], F16, tag="prow16")
    s_loc = tiny.tile([1, 1], F32, tag="s_loc")
    nc.scalar.activation(
        out=p_loc, in_=ps_f2e, func=mybir.ActivationFunctionType.Exp,
        bias=neg_cmax, scale=1.0, accum_out=s_loc,
    )

    # transpose p_loc [1,512] -> pT [128, MQ] (m = jj*128 + p)
    pT = tiny.tile([128, MQ], F16, tag="pT")
    ps_t2 = psT.tile([128, MQ], F32, tag="psT")
    for jj in range(MQ):
        nc.tensor.matmul(ps_t2[:, jj : jj + 1], p_loc[:, jj * 128 : (jj + 1) * 128], ones11,
                         start=True, stop=True)
    nc.vector.tensor_copy(pT, ps_t2)

    if stage < 14:
        _dummy_out(nc, rows, out_ep, out_st)
        return
    # ---- accep_local = p_loc @ ep_local  [1, H]  (fp16)
    ps_accep = psA.tile([1, H], F32, tag="psA")
    for jj in range(MQ):
        for n0, nn in regions768:
            nc.tensor.matmul(
                ps_accep[:, n0 : n0 + nn], pT[:, jj : jj + 1], ep_sb[:, jj, n0 : n0 + nn],
                start=(jj == 0), stop=(jj == MQ - 1),
            )

    if stage < 2:
        r = rows.tile([1, H], F32, tag="rows")
        nc.vector.tensor_copy(r, ps_accep)
        nc.gpsimd.dma_start(out=out_ep, in_=r)
        d2 = rows.tile([1, H + 2], F32, tag="row770")
        nc.vector.memset(d2, 0.0)
        nc.gpsimd.dma_start(out=out_st, in_=d2)
        return
    # pack [cmax_l, s_loc, accep] into one row and AllGather
    # payload padded to H+8 floats so per-rank AllGather offsets stay
    # 32-byte aligned
    PAD = H + 8
    row770 = rows.tile([1, PAD], F32, tag="row770")
    nc.vector.memset(row770[:, H + 2 : PAD], 0.0)
    nc.vector.tensor_copy(row770[:, 0:1], cmax_l)
    nc.vector.tensor_copy(row770[:, 1:2], s_loc)
    nc.scalar.activation(out=row770[:, 2 : H + 2], in_=ps_accep,
                         func=mybir.ActivationFunctionType.Copy)
    cc2_in = dram.tile([1, PAD], F32, tag="cc2i")
    cc2_out = dram.tile([NCORES * PAD], F32, tag="cc2o")
    nc.gpsimd.dma_start(out=cc2_in[:], in_=row770)
    if sim1 or fake_cc:
        for _r in range(NCORES):
            nc.gpsimd.dma_start(out=cc2_out[:].rearrange("(r k) -> r k", r=NCORES)[_r : _r + 1, :], in_=cc2_in[:])
    else:
        nc.gpsimd.collective_compute(
            "AllGather", mybir.AluOpType.bypass, replica_groups=RG,
            ins=[cc2_in.opt()], outs=[cc2_out.opt()],
        )
    _warm(24, c=0, off=0)
    Y = tiny.tile([NCORES, PAD], F32, tag="Y")
    nc.gpsimd.dma_start(out=Y, in_=cc2_out[:].rearrange("(r k) -> r k", r=NCORES))

    # ---- combine f2e softmax across cores (identical on every core)
    ps_cr = psT.tile([1, NCORES], F32, tag="psT")
    nc.tensor.matmul(ps_cr, Y[:, 0:1], ident[0:NCORES, 0:NCORES],
                     start=True, stop=True)
    c2 = tiny.tile([1, 1], F32, tag="c2")
    nc.vector.reduce_max(c2, ps_cr, axis=mybir.AxisListType.X)
    ps_cb = psT.tile([NCORES, 1], F32, tag="psT2")
    nc.tensor.matmul(ps_cb, ones_col[0:1, :].broadcast_to([1, NCORES]), c2, start=True, stop=True)
    dY = tiny.tile([NCORES, 1], F32, tag="dY")
    nc.vector.tensor_tensor(out=dY, in0=Y[:, 0:1], in1=ps_cb,
                            op=mybir.AluOpType.subtract)
    coef = tiny.tile([NCORES, 1], F32, tag="coef")
    nc.scalar.activation(out=coef, in_=dY, func=mybir.ActivationFunctionType.Exp)
    ps_comb = psA.tile([1, H + 1], F32, tag="psA")
    for n0, nn in [(0, 512), (512, H + 1 - 512)]:
        nc.tensor.matmul(ps_comb[:, n0 : n0 + nn], coef,
                         Y[:, 1 + n0 : 1 + n0 + nn],
                         start=True, stop=True)
    comb_row = rows.tile([1, H + 1], F32, tag="row770")
    nc.scalar.activation(out=comb_row, in_=ps_comb, func=mybir.ActivationFunctionType.Copy)
    Sinv = tiny.tile([1, 1], F32, tag="Sinv")
    nc.vector.reciprocal(Sinv, comb_row[:, 0:1])
    ep_row = rows.tile([1, H], F32, tag="rows")
    nc.vector.tensor_scalar_mul(ep_row, comb_row[:, 1 : H + 1], Sinv)
    nc.sync.dma_start(out=out_ep, in_=ep_row)

    # ---- v = W @ ep_.T  as a row: v_row = ep_ @ W.T  [1, H]  (fp16 wt)
    ep_row16 = rows.tile([1, H], F16, tag="prow16")
    nc.vector.tensor_copy(ep_row16, ep_row)
    ep_col = tiny.tile([128, HC], F16, tag="ep_col")
    ps_t3 = psT.tile([128, HC], F32, tag="psT")
    for c in range(HC):
        nc.tensor.matmul(ps_t3[:, c : c + 1], ep_row16[:, c * 128 : (c + 1) * 128], ones11,
                         start=True, stop=True)
    nc.vector.tensor_copy(ep_col, ps_t3)
    ps_v = psA.tile([1, H], F32, tag="psA")
    for c in range(HC):
        for n0, nn in regions768:
            nc.tensor.matmul(
                ps_v[:, n0 : n0 + nn], ep_col[:, c : c + 1], wt_sb[:, c, n0 : n0 + nn],
                start=(c == 0), stop=(c == HC - 1),
            )
    v_row = rows.tile([1, H], F16, tag="prow16")
    nc.scalar.activation(out=v_row, in_=ps_v, func=mybir.ActivationFunctionType.Copy)
    vT = tiny.tile([128, HC], F16, tag="vT")
    ps_t4 = psT.tile([128, HC], F32, tag="psT")
    for c in range(HC):
        nc.tensor.matmul(ps_t4[:, c : c + 1], v_row[:, c * 128 : (c + 1) * 128], ones11,
                         start=True, stop=True)
    nc.vector.tensor_copy(vT, ps_t4)

    if stage < 3:
        d2 = rows.tile([1, H + 2], F32, tag="row770")
        nc.vector.memset(d2, 0.0)
        nc.gpsimd.dma_start(out=out_st, in_=d2)
        return
    # ---- e2f local logits [1, NS] fp16, transposed to [128, NT]
    # via K=1 matmuls (n = j*128 + p)
    x2 = psT.tile([128, NT], F32, tag="psX")
    _pend = None
    for b in range(NB):
        ps_e2f = psE.tile([1, 512], F32, tag="psE")
        for c in range(HC):
            nc.tensor.matmul(
                ps_e2f,
                vT[:, c : c + 1],
                fT(c)[:, b * 512 : (b + 1) * 512],
                start=(c == 0), stop=(c == HC - 1),
            )
        if _pend is not None:
            pb, prow = _pend
            for jj in range(4):
                j = pb * 4 + jj
                nc.tensor.matmul(x2[:, j : j + 1], prow[:, jj * 128 : (jj + 1) * 128],
                                 ones_col[0:1, :], start=True, stop=True)
        e2f_row = rows.tile([1, 512], F32, tag="erow")
        nc.scalar.activation(out=e2f_row, in_=ps_e2f,
                             func=mybir.ActivationFunctionType.Copy)
        _pend = (b, e2f_row)
    pb, prow = _pend
    for jj in range(4):
        j = pb * 4 + jj
        nc.tensor.matmul(x2[:, j : j + 1], prow[:, jj * 128 : (jj + 1) * 128],
                         ones_col[0:1, :], start=True, stop=True)

    # ---- e2f softmax partials (n = p*NT + j)
    m_p = tiny.tile([128, 1], F32, tag="m_p")
    nc.vector.reduce_max(m_p, x2, axis=mybir.AxisListType.X)
    ps_mr = psT.tile([1, 128], F32, tag="psT")
    nc.tensor.matmul(ps_mr, m_p, ident, start=True, stop=True)
    Cg = tiny.tile([1, 1], F32, tag="Cg")
    negCg = tiny.tile([1, 1], F32, tag="negCg")
    nc.vector.reduce_max(Cg, ps_mr, axis=mybir.AxisListType.X)
    nc.vector.tensor_reduce(negCg, ps_mr, axis=mybir.AxisListType.X,
                            op=mybir.AluOpType.max, negate=True)
    ps_cb2 = psT.tile([128, 1], F32, tag="psT2")
    nc.tensor.matmul(ps_cb2, ones_col[0:1, :].broadcast_to([1, 128]), negCg,
                     start=True, stop=True)
    negC = tiny.tile([128, 1], F32, tag="negC")
    nc.vector.tensor_copy(negC, ps_cb2)
    s_ps = tiny.tile([128, 1], F32, tag="s_ps")
    p2 = tiny.tile([128, NT], F16, tag="p2")
    nc.scalar.activation(
        out=p2, in_=x2, func=mybir.ActivationFunctionType.Exp,
        bias=negC, scale=1.0, accum_out=s_ps,
    )
    ps_S = psT.tile([1, 1], F32, tag="psT3")
    nc.tensor.matmul(ps_S, s_ps, ones_col, start=True, stop=True)

    if stage < 4:
        d2 = rows.tile([1, H + 2], F32, tag="row770")
        nc.vector.memset(d2, 0.0)
        nc.gpsimd.dma_start(out=out_st, in_=d2)
        return
    # ---- acc = sum_n p_n * s[n, :]  (fp16 matmuls, fp32 accumulate)
    ps_acc = psA.tile([1, H], F32, tag="psA")
    for n0, nn in regions768:
        for j in range(NT):
            nc.tensor.matmul(
                ps_acc[:, n0 : n0 + nn], p2[:, j : j + 1], fnorm_sb[:, j, n0 : n0 + nn],
                start=(j == 0), stop=(j == NT - 1),
            )

    # ---- stats out: [C, S, acc]
    st_row = rows.tile([1, H + 2], F32, tag="row770")
    nc.vector.tensor_copy(st_row[:, 0:1], Cg)
    nc.vector.tensor_copy(st_row[:, 1:2], ps_S)
    nc.scalar.activation(out=st_row[:, 2 : H + 2], in_=ps_acc,
                         func=mybir.ActivationFunctionType.Copy)
    nc.sync.dma_start(out=out_st, in_=st_row)


def build(stage=999, reps=1, sim1=False, fake_cc=False):
    nc = bacc.Bacc("TRN2", target_bir_lowering=False, debug=False,
                   num_devices=1 if sim1 else NCORES)
    factT = nc.dram_tensor("factT", [H, NS], F16, kind="ExternalInput").ap()
    factn = nc.dram_tensor("factn", [NS, H], F16, kind="ExternalInput").ap()
    epT = nc.dram_tensor("epT", [H, MS], F32R, kind="ExternalInput").ap()
    epn = nc.dram_tensor("epn", [MS, H], F16, kind="ExternalInput").ap()
    w = nc.dram_tensor("w", [H, H], F32R, kind="ExternalInput").ap()
    wt = nc.dram_tensor("wt", [H, H], F16, kind="ExternalInput").ap()
    out_ep = nc.dram_tensor("out_ep", [1, H], F32, kind="ExternalOutput").ap()
    out_st = nc.dram_tensor("out_st", [1, H + 2], F32, kind="ExternalOutput").ap()
    from contextlib import ExitStack

    with tile.TileContext(nc) as tc:
        with ExitStack() as ctx:
            pools = {}
            for r in range(reps):
                _body(nc, tc, ctx, factT, factn, epT, epn, w, wt, out_ep, out_st,
                      stage=stage, pools=pools, sim1=sim1, fake_cc=fake_cc, rep=r)
    nc.compile()
    return nc


def make_in_maps(fact, elements_p, weights):
    fact = np.asarray(fact, np.float32)
    elements_p = np.asarray(elements_p, np.float32)
    weights = np.asarray(weights, np.float32)
    wt = np.ascontiguousarray(weights.T)
    s = (fact - np.float32(SHIFT)).astype(np.float16)
    in_maps = []
    for i in range(NCORES):
        ss = s[i * NS : (i + 1) * NS]
        es = elements_p[i * MS : (i + 1) * MS]
        in_maps.append({
            "factT": np.ascontiguousarray(ss.T),
            "factn": np.ascontiguousarray(ss),
            "epT": np.ascontiguousarray(es.T),
            "epn": es.astype(np.float16),
            "w": weights,
            "wt": wt.astype(np.float16),
        })
    return in_maps


def combine(results):
    st = np.stack([np.asarray(results[i]["out_st"][0], np.float64) for i in range(NCORES)])
    C = st[:, 0]
    S = st[:, 1]
    acc = st[:, 2:]
    Cg = C.max()
    coef = np.exp(C - Cg)
    fact_ = (coef[:, None] * acc).sum(0, keepdims=True) / (coef * S).sum() + SHIFT
    ep_ = np.asarray(results[0]["out_ep"], np.float32)
    return fact_.astype(np.float32), ep_


def kernel(fact, elements_p, weights, **run_kwargs):
    if "nc" not in _CACHE:
        _CACHE["nc"] = build()
    nc = _CACHE["nc"]
    in_maps = make_in_maps(fact, elements_p, weights)
    res = bass_utils.run_bass_kernel_spmd(
        nc, in_maps, core_ids=list(range(NCORES)), **run_kwargs
    )
    _CACHE["last_result"] = res
    return combine(res.results)


def build_sim1():
    return build(sim1=True)


# revision 8
# speedup vs baseline: 7.8770x; 7.8770x over previous
"""Trainium2 Bass kernel for nn_AggregationLayer (pooling) — v2.

Computation (reference):
    fact: [N,H], elements_p: [M,H], weights: [H,H]   (N=32768, M=4096, H=768)
    fact_Q = max(fact, axis=0)                     # [1,H]
    f2e = softmax(fact_Q @ W @ ep.T)               # [1,M]
    ep_ = f2e @ ep                                 # [1,H]
    e2f = softmax_n(fact @ W @ ep_.T)              # [N,1]
    fact_ = e2f.T @ fact                           # [1,H]
    returns (fact_, ep_)

v2 key ideas vs baseline:
  - fact is sent ONCE per layout as fp16 of (fact - 4.0).  The shift makes
    the column max (values ~4.05 +- 0.3) carry ~8x more absolute precision
    in fp16; the e2f logits only change by a constant (softmax-invariant)
    and the weighted row-sum just needs +4.0 added back on the host.
  - The precision-critical f2e chain (fQ -> qw -> l1) stays fp32: w and epT
    are fp32r (full-rate PE streaming for free dims >= 256).
  - factT DMAs straight into its resident [128, HC, NS] SBUF layout; the
    per-chunk column max runs on DVE as chunks land, and the cross-core
    AllReduce(max) is split in two halves pipelined behind the DMA.
  - Everything downstream (matvec, exp weights, weighted sum) is fp16 at
    1 cycle/row on PE.
Distribution: fact sharded along N (4096 rows/core), elements_p along M
(512 rows/core), weights replicated.  AllReduce(max) for fact_Q, AllGather
for the f2e softmax stats; e2f softmax partials combined on host.
"""
import sys

sys.path.insert(0, "/opt/trn_rl_repo")

import numpy as np
import ml_dtypes

import concourse.bass as bass
import concourse.tile as tile
import concourse.mybir as mybir
from concourse import bacc, bass_utils

N, M, H = 32768, 4096, 768
NCORES = 8
NS = N // NCORES          # 4096 fact rows per core
MS = M // NCORES          # 512 ep rows per core
HC = H // 128             # 6 h-chunks of 128
NB = NS // 512            # 8 blocks for e2f matvec
NT = NS // 128            # 32 row-tiles of 128
MQ = MS // 128            # 4 ep row-tiles
SHIFT = 4.0

F32 = mybir.dt.float32
F32R = mybir.dt.float32r
F16 = mybir.dt.float16
RG = [list(range(NCORES))]

_CACHE = {}


def _dummy_out(nc, rows, out_ep, out_st):
    d1 = rows.tile([1, H], F32, tag="rows")
    d2 = rows.tile([1, H + 2], F32, tag="row770")
    nc.vector.memset(d1, 0.0)
    nc.vector.memset(d2, 0.0)
    nc.gpsimd.dma_start(out=out_ep, in_=d1)
    nc.gpsimd.dma_start(out=out_st, in_=d2)


def _body(nc, tc, ctx, factT, factn, epT, epn, w, wt, out_ep, out_st, stage=999, pools=None, sim1=False, fake_cc=False, rep=0):
    par = rep % 2
    if pools is None:
        pools = {}
    def pool(name, **kw):
        if name not in pools:
            pools[name] = ctx.enter_context(tc.tile_pool(name=name, **kw))
        return pools[name]
    dram = pool("dram", bufs=1, space="DRAM")
    big = pool("big", bufs=1)
    rows = pool("rows", bufs=2)
    tiny = pool("tiny", bufs=1)
    psA = pool("psA", bufs=1, space="PSUM")
    psE = pool("psE", bufs=2, space="PSUM")
    psT = pool("psT", bufs=1, space="PSUM")
    if "consts" not in pools:
        ones11 = tiny.tile([1, 1], F16, tag="ones11")
        nc.vector.memset(ones11, 1.0)
        ones_col = tiny.tile([128, 1], F32, tag="ones_col")
        nc.vector.memset(ones_col, 1.0)
        ident_d = nc.inline_tensor(np.eye(128, dtype=np.float32), name="ident128")
        ident_t = tiny.tile([128, 128], F32, tag="ident128")
        nc.sync.dma_start(out=ident_t, in_=ident_d.ap())
        ones_col16 = tiny.tile([128, 1], F16, tag="ones_col16")
        nc.vector.memset(ones_col16, 1.0)
        shift_col = tiny.tile([128, 1], F32, tag="shift_col")
        nc.vector.memset(shift_col, SHIFT)
        pools["consts"] = (ones11, ones_col, ident_t, ones_col16, shift_col)
    ones11, ones_col, ident, ones_col16, shift_col = pools["consts"]

    # ---- fact loads: factT fp16 chunks DMA directly into the resident
    # [128, HC, NS] layout on the SP HWDGE ring; each chunk's column max
    # runs on DVE as it lands.
    # chunks 0-4 double-buffered by rep parity (next rep's DMA + col-max +
    # AllReduce overlap this rep's matvec/wsum); chunk 5 single-buffered
    # (its reload chain hides under the weighted sum) to fit SBUF.
    factT_db = big.tile([128, HC - 1, NS], F16, tag=f"factT{par}")
    factT_sh = big.tile([128, 1, NS], F16, tag="factT_sh")
    def fT(c):
        return factT_db[:, c, :] if c < HC - 1 else factT_sh[:, 0, :]
    factT_d = factT.rearrange("(c p) n -> p c n", p=128)
    fQl = tiny.tile([128, HC], F32, tag=f"fQl{par}")
    _fact_dmas = []
    for c in range(HC):
        d = nc.sync.dma_start(out=fT(c), in_=factT_d[:, c, :])
        _fact_dmas.append(d)
        nc.vector.reduce_max(fQl[:, c : c + 1], fT(c), axis=mybir.AxisListType.X)

    # ---- small input loads on the ACT HWDGE ring, gated behind the first
    # few factT chunks so factT streams at full rate early; then fnorm.
    smw = pool("smw", bufs=1)
    smwt = pool("smwt", bufs=1)
    smept = pool("smept", bufs=1)
    smep = pool("smep", bufs=1)
    w_d = w.rearrange("(c p) j -> p c j", p=128)
    wt_d = wt.rearrange("(c p) j -> p c j", p=128)
    epT_d = epT.rearrange("(c p) m -> p c m", p=128)
    epn_d = epn.rearrange("(j p) h -> p j h", p=128)
    _gate = _fact_dmas[-1].ins
    _prev_small = None
    def _small_dma(dst, src):
        nonlocal _prev_small
        d = nc.scalar.dma_start(out=dst, in_=src)
        if _prev_small is None:
            tile.add_dep_helper(d.ins, _gate, sync=True, reason="factT first")
        else:
            tile.add_dep_helper(d.ins, _prev_small, sync=False, reason="ACT ring order")
        _prev_small = d.ins
        return d
    w_sb = smw.tile([128, HC, H], F32R, tag="w")
    for c in range(HC):
        _small_dma(w_sb[:, c, :], w_d[:, c, :])
    epT_sb = smept.tile([128, HC, MS], F32R, tag="epT")
    _small_dma(epT_sb, epT_d)
    ep_sb = smep.tile([128, MQ, H], F16, tag="ep")
    _small_dma(ep_sb, epn_d)
    wt_sb = smwt.tile([128, HC, H], F16, tag="wt")
    for c in range(HC):
        _small_dma(wt_sb[:, c, :], wt_d[:, c, :])
    # fnorm (row-major fp16 fact) streams last, in quarters so the final
    # weighted sum can start before the whole thing lands.
    fnorm_sb = big.tile([128, NT, H], F16, tag="fnorm")
    _fn_d = factn.rearrange("(j p) h -> p j h", p=128)
    fn_dmas = []
    for q in range(4):
        fn_dmas.append(_small_dma(fnorm_sb[:, q * 8 : (q + 1) * 8, :], _fn_d[:, q * 8 : (q + 1) * 8, :]))

    if stage < 1:
        _dummy_out(nc, rows, out_ep, out_st)
        return
    # ---- PE warmup (clock ramp) against landed factT chunks
    def _warm(n, c=0, off=0):
        for _ in range(n):
            ps_w = psE.tile([1, 512], F32, tag="psE")
            nc.tensor.matmul(ps_w, ones_col16, fT(c)[:, off : off + 512],
                             start=True, stop=True)
    # With factT double-buffered, the next rep's AllReduce overlaps this
    # rep's PE tail, so only a small burst is needed to bridge any gap
    # (plus rep-0 cold start, which the slope timing cancels).
    for k in range(6):
        _warm(1, c=k, off=0)
    _warm(8, c=HC - 1, off=2048)

    if stage < 11:
        _dummy_out(nc, rows, out_ep, out_st)
        return
    # ---- AllReduce(max) of the local column max (single [768] op)
    fQg = tiny.tile([128, HC], F32, tag=f"fQg{par}")
    cc1_in = dram.tile([H], F32, tag=f"cc1i{par}")
    cc1_out = dram.tile([H], F32, tag=f"cc1o{par}")
    nc.gpsimd.dma_start(out=cc1_in[:].rearrange("(c p) -> p c", p=128), in_=fQl)
    if sim1 or fake_cc:
        nc.gpsimd.dma_start(out=cc1_out[:], in_=cc1_in[:])
    else:
        nc.gpsimd.collective_compute(
            "AllReduce", mybir.AluOpType.max, replica_groups=RG,
            ins=[cc1_in.opt()], outs=[cc1_out.opt()],
        )
    nc.gpsimd.dma_start(
        out=fQg, in_=cc1_out[:].rearrange("(c p) -> p c", p=128))
    # undo the host-side shift: fQ = smax + 4.0, in fp32r for the qw matmul
    fQr = tiny.tile([128, HC], F32R, tag=f"fQr{par}")
    nc.scalar.activation(out=fQr, in_=fQg, func=mybir.ActivationFunctionType.Copy,
                         bias=float(SHIFT), scale=1.0)

    if stage < 12:
        _dummy_out(nc, rows, out_ep, out_st)
        return
    regions768 = [(0, 512), (512, 256)]

    # ---- qw = fact_Q @ W  (fp32r at full rate; contraction over h)
    ps_qw = psA.tile([1, H], F32, tag="psA")
    for c in range(HC):
        for n0, nn in regions768:
            nc.tensor.matmul(
                ps_qw[:, n0 : n0 + nn], fQr[:, c : c + 1], w_sb[:, c, n0 : n0 + nn],
                start=(c == 0), stop=(c == HC - 1),
            )
    qw_row = rows.tile([1, H], F32, tag="rows")
    nc.scalar.activation(out=qw_row, in_=ps_qw, func=mybir.ActivationFunctionType.Copy)

    # transpose qw row -> [128, HC] via K=1 matmuls against ones
    # (fp32: tiny, 4 cy/row is fine here)
    ones11_32 = None
    qwT = tiny.tile([128, HC], F32R, tag="qwT")
    ps_t = psT.tile([128, HC], F32, tag="psT")
    for c in range(HC):
        nc.tensor.matmul(ps_t[:, c : c + 1], qw_row[:, c * 128 : (c + 1) * 128],
                         ones_col[0:1, :], start=True, stop=True)
    nc.vector.tensor_copy(qwT, ps_t)

    if stage < 13:
        _dummy_out(nc, rows, out_ep, out_st)
        return
    # ---- f2e local logits [1, MS] (fp32r)
    ps_f2e = psA.tile([1, MS], F32, tag="psA")
    for c in range(HC):
        nc.tensor.matmul(
            ps_f2e, qwT[:, c : c + 1], epT_sb[:, c, :],
            start=(c == 0), stop=(c == HC - 1),
        )
    # local softmax pieces
    cmax_l = tiny.tile([1, 1], F32, tag="cmax_l")
    neg_cmax = tiny.tile([1, 1], F32, tag="neg_cmax")
    nc.vector.reduce_max(cmax_l, ps_f2e, axis=mybir.AxisListType.X)
    nc.vector.tensor_reduce(neg_cmax, ps_f2e, axis=mybir.AxisListType.X,
                            op=mybir.AluOpType.max, negate=True)
    p_loc = rows.tile([1, MS], F16, tag="prow16")
    s_loc = tiny.tile([1, 1], F32, tag="s_loc")
    nc.scalar.activation(
        out=p_loc, in_=ps_f2e, func=mybir.ActivationFunctionType.Exp,
        bias=neg_cmax, scale=1.0, accum_out=s_loc,
    )

    # transpose p_loc [1,512] -> pT [128, MQ] (m = jj*128 + p)
    pT = tiny.tile([128, MQ], F16, tag="pT")
    ps_t2 = psT.tile([128, MQ], F32, tag="psT")
    for jj in range(MQ):
        nc.tensor.matmul(ps_t2[:, jj : jj + 1], p_loc[:, jj * 128 : (jj + 1) * 128], ones11,
                         start=True, stop=True)
    nc.vector.tensor_copy(pT, ps_t2)

    if stage < 14:
        _dummy_out(nc, rows, out_ep, out_st)
        return
    # ---- accep_local = p_loc @ ep_local  [1, H]  (fp16)
    ps_accep = psA.tile([1, H], F32, tag="psA")
    for jj in range(MQ):
        for n0, nn in regions768:
            nc.tensor.matmul(
                ps_accep[:, n0 : n0 + nn], pT[:, jj : jj + 1], ep_sb[:, jj, n0 : n0 + nn],
                start=(jj == 0), stop=(jj == MQ - 1),
            )

    if stage < 2:
        r = rows.tile([1, H], F32, tag="rows")
        nc.vector.tensor_copy(r, ps_accep)
        nc.gpsimd.dma_start(out=out_ep, in_=r)
        d2 = rows.tile([1, H + 2], F32, tag="row770")
        nc.vector.memset(d2, 0.0)
        nc.gpsimd.dma_start(out=out_st, in_=d2)
        return
    # pack [cmax_l, s_loc, accep] into one row and AllGather
    # payload padded to H+8 floats so per-rank AllGather offsets stay
    # 32-byte aligned
    PAD = H + 8
    row770 = rows.tile([1, PAD], F32, tag="row770")
    nc.vector.memset(row770[:, H + 2 : PAD], 0.0)
    nc.vector.tensor_copy(row770[:, 0:1], cmax_l)
    nc.vector.tensor_copy(row770[:, 1:2], s_loc)
    nc.scalar.activation(out=row770[:, 2 : H + 2], in_=ps_accep,
                         func=mybir.ActivationFunctionType.Copy)
    cc2_in = dram.tile([1, PAD], F32, tag="cc2i")
    cc2_out = dram.tile([NCORES * PAD], F32, tag="cc2o")
    nc.gpsimd.dma_start(out=cc2_in[:], in_=row770)
    if sim1 or fake_cc:
        for _r in range(NCORES):
            nc.gpsimd.dma_start(out=cc2_out[:].rearrange("(r k) -> r k", r=NCORES)[_r : _r + 1, :], in_=cc2_in[:])
    else:
        nc.gpsimd.collective_compute(
            "AllGather", mybir.AluOpType.bypass, replica_groups=RG,
            ins=[cc2_in.opt()], outs=[cc2_out.opt()],
        )
    _warm(30, c=0, off=0)
    Y = tiny.tile([NCORES, PAD], F32, tag="Y")
    nc.gpsimd.dma_start(out=Y, in_=cc2_out[:].rearrange("(r k) -> r k", r=NCORES))

    # ---- combine f2e softmax across cores (identical on every core)
    ps_cr = psT.tile([1, NCORES], F32, tag="psT")
    nc.tensor.matmul(ps_cr, Y[:, 0:1], ident[0:NCORES, 0:NCORES],
                     start=True, stop=True)
    c2 = tiny.tile([1, 1], F32, tag="c2")
    nc.vector.reduce_max(c2, ps_cr, axis=mybir.AxisListType.X)
    ps_cb = psT.tile([NCORES, 1], F32, tag="psT2")
    nc.tensor.matmul(ps_cb, ones_col[0:1, :].broadcast_to([1, NCORES]), c2, start=True, stop=True)
    dY = tiny.tile([NCORES, 1], F32, tag="dY")
    nc.vector.tensor_tensor(out=dY, in0=Y[:, 0:1], in1=ps_cb,
                            op=mybir.AluOpType.subtract)
    coef = tiny.tile([NCORES, 1], F32, tag="coef")
    nc.scalar.activation(out=coef, in_=dY, func=mybir.ActivationFunctionType.Exp)
    ps_comb = psA.tile([1, H + 1], F32, tag="psA")
    for n0, nn in [(0, 512), (512, H + 1 - 512)]:
        nc.tensor.matmul(ps_comb[:, n0 : n0 + nn], coef,
                         Y[:, 1 + n0 : 1 + n0 + nn],
                         start=True, stop=True)
    comb_row = rows.tile([1, H + 1], F32, tag="row770")
    nc.scalar.activation(out=comb_row, in_=ps_comb, func=mybir.ActivationFunctionType.Copy)
    Sinv = tiny.tile([1, 1], F32, tag="Sinv")
    nc.vector.reciprocal(Sinv, comb_row[:, 0:1])
    ep_row = rows.tile([1, H], F32, tag="rows")
    nc.vector.tensor_scalar_mul(ep_row, comb_row[:, 1 : H + 1], Sinv)
    nc.sync.dma_start(out=out_ep, in_=ep_row)

    # ---- v = W @ ep_.T  as a row: v_row = ep_ @ W.T  [1, H]  (fp16 wt)
    ep_row16 = rows.tile([1, H], F16, tag="prow16")
    nc.vector.tensor_copy(ep_row16, ep_row)
    ep_col = tiny.tile([128, HC], F16, tag="ep_col")
    ps_t3 = psT.tile([128, HC], F32, tag="psT")
    for c in range(HC):
        nc.tensor.matmul(ps_t3[:, c : c + 1], ep_row16[:, c * 128 : (c + 1) * 128], ones11,
                         start=True, stop=True)
    nc.vector.tensor_copy(ep_col, ps_t3)
    ps_v = psA.tile([1, H], F32, tag="psA")
    for c in range(HC):
        for n0, nn in regions768:
            nc.tensor.matmul(
                ps_v[:, n0 : n0 + nn], ep_col[:, c : c + 1], wt_sb[:, c, n0 : n0 + nn],
                start=(c == 0), stop=(c == HC - 1),
            )
    v_row = rows.tile([1, H], F16, tag="prow16")
    nc.scalar.activation(out=v_row, in_=ps_v, func=mybir.ActivationFunctionType.Copy)
    vT = tiny.tile([128, HC], F16, tag="vT")
    ps_t4 = psT.tile([128, HC], F32, tag="psT")
    for c in range(HC):
        nc.tensor.matmul(ps_t4[:, c : c + 1], v_row[:, c * 128 : (c + 1) * 128], ones11,
                         start=True, stop=True)
    nc.vector.tensor_copy(vT, ps_t4)

    if stage < 3:
        d2 = rows.tile([1, H + 2], F32, tag="row770")
        nc.vector.memset(d2, 0.0)
        nc.gpsimd.dma_start(out=out_st, in_=d2)
        return
    # ---- e2f local logits [1, NS] fp16, transposed to [128, NT]
    # via K=1 matmuls (n = j*128 + p)
    x2 = psT.tile([128, NT], F32, tag="psX")
    _pend = None
    for b in range(NB):
        ps_e2f = psE.tile([1, 512], F32, tag="psE")
        for c in range(HC):
            nc.tensor.matmul(
                ps_e2f,
                vT[:, c : c + 1],
                fT(c)[:, b * 512 : (b + 1) * 512],
                start=(c == 0), stop=(c == HC - 1),
            )
        if _pend is not None:
            pb, prow = _pend
            for jj in range(4):
                j = pb * 4 + jj
                nc.tensor.matmul(x2[:, j : j + 1], prow[:, jj * 128 : (jj + 1) * 128],
                                 ones_col[0:1, :], start=True, stop=True)
        e2f_row = rows.tile([1, 512], F32, tag="erow")
        nc.scalar.activation(out=e2f_row, in_=ps_e2f,
                             func=mybir.ActivationFunctionType.Copy)
        _pend = (b, e2f_row)
    pb, prow = _pend
    for jj in range(4):
        j = pb * 4 + jj
        nc.tensor.matmul(x2[:, j : j + 1], prow[:, jj * 128 : (jj + 1) * 128],
                         ones_col[0:1, :], start=True, stop=True)

    # ---- e2f softmax partials (n = p*NT + j)
    m_p = tiny.tile([128, 1], F32, tag="m_p")
    nc.vector.reduce_max(m_p, x2, axis=mybir.AxisListType.X)
    ps_mr = psT.tile([1, 128], F32, tag="psT")
    nc.tensor.matmul(ps_mr, m_p, ident, start=True, stop=True)
    Cg = tiny.tile([1, 1], F32, tag="Cg")
    negCg = tiny.tile([1, 1], F32, tag="negCg")
    nc.vector.reduce_max(Cg, ps_mr, axis=mybir.AxisListType.X)
    nc.vector.tensor_reduce(negCg, ps_mr, axis=mybir.AxisListType.X,
                            op=mybir.AluOpType.max, negate=True)
    ps_cb2 = psT.tile([128, 1], F32, tag="psT2")
    nc.tensor.matmul(ps_cb2, ones_col[0:1, :].broadcast_to([1, 128]), negCg,
                     start=True, stop=True)
    negC = tiny.tile([128, 1], F32, tag="negC")
    nc.vector.tensor_copy(negC, ps_cb2)
    s_ps = tiny.tile([128, 1], F32, tag="s_ps")
    p2 = tiny.tile([128, NT], F16, tag="p2")
    nc.scalar.activation(
        out=p2, in_=x2, func=mybir.ActivationFunctionType.Exp,
        bias=negC, scale=1.0, accum_out=s_ps,
    )
    ps_S = psT.tile([1, 1], F32, tag="psT3")
    nc.tensor.matmul(ps_S, s_ps, ones_col, start=True, stop=True)

    if stage < 4:
        d2 = rows.tile([1, H + 2], F32, tag="row770")
        nc.vector.memset(d2, 0.0)
        nc.gpsimd.dma_start(out=out_st, in_=d2)
        return
    # ---- acc = sum_n p_n * s[n, :]  (fp16 matmuls, fp32 accumulate)
    ps_acc = psA.tile([1, H], F32, tag="psA")
    for n0, nn in regions768:
        for j in range(NT):
            nc.tensor.matmul(
                ps_acc[:, n0 : n0 + nn], p2[:, j : j + 1], fnorm_sb[:, j, n0 : n0 + nn],
                start=(j == 0), stop=(j == NT - 1),
            )

    # ---- stats out: [C, S, acc]
    st_row = rows.tile([1, H + 2], F32, tag="row770")
    nc.vector.tensor_copy(st_row[:, 0:1], Cg)
    nc.vector.tensor_copy(st_row[:, 1:2], ps_S)
    nc.scalar.activation(out=st_row[:, 2 : H + 2], in_=ps_acc,
                         func=mybir.ActivationFunctionType.Copy)
    nc.sync.dma_start(out=out_st, in_=st_row)


def build(stage=999, reps=1, sim1=False, fake_cc=False):
    nc = bacc.Bacc("TRN2", target_bir_lowering=False, debug=False,
                   num_devices=1 if sim1 else NCORES)
    factT = nc.dram_tensor("factT", [H, NS], F16, kind="ExternalInput").ap()
    factn = nc.dram_tensor("factn", [NS, H], F16, kind="ExternalInput").ap()
    epT = nc.dram_tensor("epT", [H, MS], F32R, kind="ExternalInput").ap()
    epn = nc.dram_tensor("epn", [MS, H], F16, kind="ExternalInput").ap()
    w = nc.dram_tensor("w", [H, H], F32R, kind="ExternalInput").ap()
    wt = nc.dram_tensor("wt", [H, H], F16, kind="ExternalInput").ap()
    out_ep = nc.dram_tensor("out_ep", [1, H], F32, kind="ExternalOutput").ap()
    out_st = nc.dram_tensor("out_st", [1, H + 2], F32, kind="ExternalOutput").ap()
    from contextlib import ExitStack

    with tile.TileContext(nc) as tc:
        with ExitStack() as ctx:
            pools = {}
            for r in range(reps):
                _body(nc, tc, ctx, factT, factn, epT, epn, w, wt, out_ep, out_st,
                      stage=stage, pools=pools, sim1=sim1, fake_cc=fake_cc, rep=r)
    nc.compile()
    return nc


def make_in_maps(fact, elements_p, weights):
    fact = np.asarray(fact, np.float32)
    elements_p = np.asarray(elements_p, np.float32)
    weights = np.asarray(weights, np.float32)
    wt = np.ascontiguousarray(weights.T)
    s = (fact - np.float32(SHIFT)).astype(np.float16)
    in_maps = []
    for i in range(NCORES):
        ss = s[i * NS : (i + 1) * NS]
        es = elements_p[i * MS : (i + 1) * MS]
        in_maps.append({
            "factT": np.ascontiguousarray(ss.T),
            "factn": np.ascontiguousarray(ss),
            "epT": np.ascontiguousarray(es.T),
            "epn": es.astype(np.float16),
            "w": weights,
            "wt": wt.astype(np.float16),
        })
    return in_maps


def combine(results):
    st = np.stack([np.asarray(results[i]["out_st"][0], np.float64) for i in range(NCORES)])
    C = st[:, 0]
    S = st[:, 1]
    acc = st[:, 2:]
    Cg = C.max()
    coef = np.exp(C - Cg)
    fact_ = (coef[:, None] * acc).sum(0, keepdims=True) / (coef * S).sum() + SHIFT
    ep_ = np.asarray(results[0]["out_ep"], np.float32)
    return fact_.astype(np.float32), ep_


def kernel(fact, elements_p, weights, **run_kwargs):
    if "nc" not in _CACHE:
        _CACHE["nc"] = build()
    nc = _CACHE["nc"]
    in_maps = make_in_maps(fact, elements_p, weights)
    res = bass_utils.run_bass_kernel_spmd(
        nc, in_maps, core_ids=list(range(NCORES)), **run_kwargs
    )
    _CACHE["last_result"] = res
    return combine(res.results)


def build_sim1():
    return build(sim1=True)
